# revision 3
# baseline (speedup 1.0000x reference)
"""Trainium2 Bass kernel for nn_CausalSE: causal cumulative-average pooling
+ squeeze-excite gating, data-parallel over batch (one NeuronCore per batch
element).

Reference math per batch element (D=512, T=8192, chunk=16, Tc=512):
    avg    = cumsum(x, t) / (t+1)
    pooled = avg[:, 15::16]                          # [D, Tc]
    h      = relu(w1 @ pooled + b1)                  # [64, Tc]
    g      = sigmoid(w2 @ h + b2)                    # [D, Tc]
    out    = repeat(g, 16, t)[:, :T] * x

The kernel is HBM-bound: per core it streams x in and out once.  x
crosses HBM as fp16 (host converts both ways), so DMA is ~16.8 MB at
the ~360-420 GB/s per-core rate => ~42-47us, and every compute engine
must stay below that.

v2 restructure (v1 was DVE-bound at ~55us busy):
  - The chunk sums moved off DVE onto the idle PE.  Because
    w1 @ (chunk_sum(x)) == chunk_sum(w1 @ x), the PE computes
    Y = w1 @ x directly from the resident fp16 x tiles (4 ki
    accumulation steps into PSUM [64, 512] banks), and DVE only
    windowed-reduces the 8x-smaller Y.  DVE busy drops to ~30us
    (gate multiplies ~18us + reduces ~10us + small scan/scale ops).
  - relu rides DVE as tensor_scalar(add b1, max 0) so ACT needs only
    the sigmoid table set (one ACT_TABLE_LOAD instead of two).
  - Loads are consolidated: one DMA per t-block carrying all 4
    d-tiles (1-2 MB per transfer) on the ACT HWDGE ring; stores go on
    the SP ring (d01/d23 pairs).  GpSimd issues nothing (its SWDGE
    descgen and multi-us drains are dead weight).
  - Engine layout per t-block: PE does the 4x(TB/512) Y-matmuls
    (ki-outer so PSUM accumulation interleaves across banks) then the
    4 gate matmuls; DVE does windowed reduces, the causal scan, scale
    + bias+relu, and the 4 gate multiplies (deferred one block, so the
    in-order DVE queue never waits on ACT); ACT does the fused
    sigmoid + 16x upsample (broadcast PSUM read) and issues loads.
"""

import sys

for _p in ("/opt/trn_rl_repo",):
    if _p not in sys.path:
        sys.path.insert(0, _p)

import numpy as np

B, D, T = 8, 512, 8192
DH = 64          # bottleneck dim = D // 8
CS = 16          # chunksize
TC = T // CS     # 512 chunks
NCORES = 8
NDT = D // 128   # 4 partition tiles of x / out
SB = 512         # Y-matmul sub-block (one PSUM bank of fp32)
CSB = SB // CS   # 32 chunk columns per sub-block
TBLOCKS = [(0, 1024), (1024, 2048), (3072, 2048), (5120, 2048),
           (7168, 1024)]
TBMAX = 2048

_compiled_nc = None


def build_nc():
    import concourse.tile as tile
    from concourse import bacc, mybir

    f32 = mybir.dt.float32
    f16 = mybir.dt.float16
    AF = mybir.ActivationFunctionType
    ALU = mybir.AluOpType
    AX = mybir.AxisListType

    # Bacc (not plain Bass): its finalize() runs the TRN2 sync-wait
    # legalization (move_matmul_waits_to_ldweights / event-semaphore
    # splitting) that walrus codegen requires.
    nc = bacc.Bacc("TRN2", target_bir_lowering=False)
    x_d = nc.declare_dram_parameter("x", [D, T], f16, isOutput=False)
    w1t_d = nc.declare_dram_parameter("w1t", [D, DH], f16, isOutput=False)
    b1_d = nc.declare_dram_parameter("b1", [DH], f32, isOutput=False)
    w2t_d = nc.declare_dram_parameter("w2t", [DH, D], f16, isOutput=False)
    b2_d = nc.declare_dram_parameter("b2", [D], f32, isOutput=False)
    scale_d = nc.declare_dram_parameter("scale", [DH, TC], f32, isOutput=False)
    out_d = nc.declare_dram_parameter("out", [D, T], f16, isOutput=True)

    with tile.TileContext(nc) as tc:
        with (
            tc.tile_pool(name="xres", bufs=1) as xres,
            tc.tile_pool(name="small", bufs=1) as small,
            tc.tile_pool(name="ups", bufs=2) as ups,
            tc.tile_pool(name="psum_y", bufs=1, space="PSUM") as psum_y,
            tc.tile_pool(name="psum_g", bufs=4, space="PSUM") as psum_g,
        ):
            # x resident in SBUF: [128, 4, 8192] fp16 = 8 MB
            xt = xres.tile([128, NDT, T], f16, tag="x", name="x")
            w1s = small.tile([128, NDT, DH], f16, tag="w1")
            w2s = small.tile([DH, D], f16, tag="w2")
            b1s = small.tile([DH, 1], f32, tag="b1")
            b2s = small.tile([128, NDT], f32, tag="b2")
            scl = small.tile([DH, TC], f32, tag="scl")
            q = small.tile([DH, TC], f32, tag="q")      # per-chunk w1@x sums
            qs = small.tile([DH, TC], f32, tag="qs")    # causal prefix
            h32 = small.tile([DH, TC], f32, tag="h32")
            h16 = small.tile([DH, TC], f16, tag="h16")
            yp = [
                psum_y.tile([DH, SB], f32, tag=f"y{sb}", name=f"y{sb}")
                for sb in range(TBMAX // SB)
            ]

            # ACT HWDGE ring: block-0 x load first (compute ramp), then the
            # replicated weights, then the remaining x loads.  The SP ring
            # carries only stores, so stores never queue behind loads.
            def load_block(t0, TB):
                nc.scalar.dma_start(
                    xt[:, :, t0:t0 + TB],
                    x_d[:, t0:t0 + TB].rearrange("(k p) t -> p k t", p=128),
                )

            load_block(*TBLOCKS[0])
            nc.scalar.dma_start(
                w1s[:], w1t_d[:].rearrange("(d p) h -> p d h", d=NDT)
            )
            nc.scalar.dma_start(b1s[:], b1_d[:].unsqueeze(1))
            load_block(*TBLOCKS[1])
            nc.scalar.dma_start(scl[:], scale_d[:])
            nc.scalar.dma_start(w2s[:], w2t_d[:])
            nc.scalar.dma_start(
                b2s[:], b2_d[:].rearrange("(d p) -> p d", d=NDT)
            )
            for t0, TB in TBLOCKS[2:]:
                load_block(t0, TB)

            # Causal pipeline: gate for chunk c needs only x[:, :16(c+1)].
            # Block k's gate multiplies + stores are emitted AFTER block
            # k+1's reduce/scan stage so the in-order DVE queue never stalls
            # waiting for the ACT sigmoid-upsample (software pipelining).
            deferred = None

            def emit_mults(items, tail=False):
                for di, t0_, TB_, u_ in items:
                    xv = xt[:, di, t0_:t0_ + TB_]
                    nc.vector.tensor_tensor(xv, xv, u_[:, :TB_], op=ALU.mult)
                    if tail:
                        # tail: per-d-tile stores, alternating rings, right
                        # behind each multiply so the drain pipelines
                        deng = nc.sync if di < 2 else nc.scalar
                        deng.dma_start(
                            out_d[di * 128:(di + 1) * 128, t0_:t0_ + TB_], xv
                        )
                if not tail:
                    di, t0_, TB_, _ = items[0]
                    for half in range(2):
                        nc.sync.dma_start(
                            out_d[half * 256:(half + 1) * 256,
                                  t0_:t0_ + TB_].rearrange(
                                      "(k p) t -> p k t", p=128),
                            xt[:, 2 * half:2 * half + 2, t0_:t0_ + TB_],
                        )

            for tb, (t0, TB) in enumerate(TBLOCKS):
                CB = TB // CS
                c0 = t0 // CS
                nsb = TB // SB
                # Y = w1 @ x for this block: ki-outer so each stationary
                # w1-slice is reused across the sub-blocks while PSUM
                # accumulation groups interleave across banks.
                for ki in range(NDT):
                    for sb in range(nsb):
                        ts = t0 + sb * SB
                        nc.tensor.matmul(
                            yp[sb][:],
                            w1s[:, ki, :],
                            xt[:, ki, ts:ts + SB],
                            start=(ki == 0),
                            stop=(ki == NDT - 1),
                        )
                # chunk sums of Y: windowed reduce straight off PSUM
                for sb in range(nsb):
                    cc = c0 + sb * CSB
                    nc.vector.reduce_sum(
                        q[:, cc:cc + CSB],
                        yp[sb][:].rearrange("p (c j) -> p c j", j=CS),
                        axis=AX.X,
                    )
                # running causal prefix over this block (carry = last col)
                nc.vector.tensor_tensor_scan(
                    qs[:, c0:c0 + CB],
                    q[:, c0:c0 + CB],
                    q[:, c0:c0 + CB],
                    0.0 if tb == 0 else qs[:, c0 - 1:c0],
                    op0=ALU.add,
                    op1=ALU.bypass,
                )
                # h = relu(prefix * 1/(16(c+1)) + b1), relu on DVE so ACT
                # only ever needs the sigmoid table set
                nc.vector.tensor_mul(
                    h32[:, c0:c0 + CB], qs[:, c0:c0 + CB], scl[:, c0:c0 + CB]
                )
                nc.vector.tensor_scalar(
                    h16[:, c0:c0 + CB], h32[:, c0:c0 + CB],
                    b1s[:, :1], 0.0, op0=ALU.add, op1=ALU.max,
                )
                last = tb == len(TBLOCKS) - 1
                if last and deferred is not None:
                    # flush the previous block's multiplies first so the
                    # tail drains in order
                    emit_mults(deferred)
                    deferred = None
                cur = []
                for di in range(NDT):
                    gp = psum_g.tile([128, TBMAX // CS], f32, tag="g",
                                     name="gp")
                    nc.tensor.matmul(
                        gp[:, :CB],
                        w2s[:, di * 128:(di + 1) * 128],
                        h16[:, c0:c0 + CB],
                        start=True,
                        stop=True,
                    )
                    # fused sigmoid + 16x upsample: broadcast-read the
                    # PSUM column per chunk, write the dense fp16 gate
                    u = ups.tile(
                        [128, TBMAX], f16, tag=f"u{di}", name=f"u{di}"
                    )
                    nc.scalar.activation(
                        u[:, :TB].rearrange("p (c j) -> p c j", j=CS),
                        gp[:, :CB].unsqueeze(2).broadcast_to([128, CB, CS]),
                        AF.Sigmoid,
                        bias=b2s[:, di:di + 1],
                    )
                    if last:
                        # tail block: multiply right behind each sigmoid so
                        # the drain pipelines at d-tile granularity
                        emit_mults([(di, t0, TB, u)], tail=True)
                    else:
                        cur.append((di, t0, TB, u))
                if deferred is not None:
                    emit_mults(deferred)
                deferred = cur if not last else None
    # run_bass_via_pjrt serializes nc.m as-is; Bacc defers register
    # allocation and TRN2 sync-wait legalization to finalize(), so it must
    # run here or walrus rejects the BIR.
    nc.finalize()
    return nc


def _host_inputs(x, w1, b1, w2, b2, chunksize):
    x = np.asarray(x)
    w1 = np.asarray(w1, dtype=np.float32)
    b1 = np.ascontiguousarray(np.asarray(b1, dtype=np.float32))
    w2 = np.asarray(w2, dtype=np.float32)
    b2 = np.ascontiguousarray(np.asarray(b2, dtype=np.float32))
    cs = int(chunksize)
    assert cs == CS and x.shape == (B, D, T), (cs, x.shape)
    x16 = np.ascontiguousarray(x.astype(np.float16))
    w1t = np.ascontiguousarray(w1.T.astype(np.float16))      # [D, DH]
    w2t = np.ascontiguousarray(w2.T.astype(np.float16))      # [DH, D]
    scale = np.broadcast_to(
        1.0 / (CS * np.arange(1, TC + 1, dtype=np.float32)), (DH, TC)
    )
    scale = np.ascontiguousarray(scale)
    shared = dict(w1t=w1t, b1=b1, w2t=w2t, b2=b2, scale=scale)
    return x16, shared


def kernel(x, w1, b1, w2, b2, chunksize):
    global _compiled_nc
    from concourse.bass_utils import run_bass_kernel_spmd

    x16, shared = _host_inputs(x, w1, b1, w2, b2, chunksize)
    if _compiled_nc is None:
        _compiled_nc = build_nc()
    in_maps = [
        {"x": np.ascontiguousarray(x16[i]), **shared} for i in range(NCORES)
    ]
    res = run_bass_kernel_spmd(_compiled_nc, in_maps, list(range(NCORES)))
    out = np.stack(
        [res.results[i]["out"] for i in range(NCORES)], axis=0
    ).astype(np.float32)
    return out


# revision 6
# speedup vs baseline: 1.0593x; 1.0593x over previous
"""Trainium2 Bass kernel for nn_CausalSE: causal cumulative-average pooling
+ squeeze-excite gating, data-parallel over batch (one NeuronCore per batch
element).

Reference math per batch element (D=512, T=8192, chunk=16, Tc=512):
    avg    = cumsum(x, t) / (t+1)
    pooled = avg[:, 15::16]                          # [D, Tc]
    h      = relu(w1 @ pooled + b1)                  # [64, Tc]
    g      = sigmoid(w2 @ h + b2)                    # [D, Tc]
    out    = repeat(g, 16, t)[:, :T] * x

The kernel is HBM-bound: per core it streams x in and out once.  x
crosses HBM as fp16 (host converts both ways), so DMA is ~16.8 MB at
the ~360-420 GB/s per-core rate => ~42-47us, and every compute engine
must stay below that.

v2 restructure (v1 was DVE-bound at ~55us busy):
  - The chunk sums moved off DVE onto the idle PE.  Because
    w1 @ (chunk_sum(x)) == chunk_sum(w1 @ x), the PE computes
    Y = w1 @ x directly from the resident fp16 x tiles (4 ki
    accumulation steps into PSUM [64, 512] banks), and DVE only
    windowed-reduces the 8x-smaller Y.  DVE busy drops to ~30us
    (gate multiplies ~18us + reduces ~10us + small scan/scale ops).
  - relu rides DVE as tensor_scalar(add b1, max 0) so ACT needs only
    the sigmoid table set (one ACT_TABLE_LOAD instead of two).
  - Loads are consolidated: one DMA per t-block carrying all 4
    d-tiles (1-2 MB per transfer) on the ACT HWDGE ring; stores go on
    the SP ring (d01/d23 pairs).  GpSimd issues nothing (its SWDGE
    descgen and multi-us drains are dead weight).
  - Engine layout per t-block: PE does the 4x(TB/512) Y-matmuls
    (ki-outer so PSUM accumulation interleaves across banks) then the
    4 gate matmuls; DVE does windowed reduces, the causal scan, scale
    + bias+relu, and the 4 gate multiplies (deferred one block, so the
    in-order DVE queue never waits on ACT); ACT does the fused
    sigmoid + 16x upsample (broadcast PSUM read) and issues loads.
"""

import sys

for _p in ("/opt/trn_rl_repo",):
    if _p not in sys.path:
        sys.path.insert(0, _p)

import numpy as np

B, D, T = 8, 512, 8192
DH = 64          # bottleneck dim = D // 8
CS = 16          # chunksize
TC = T // CS     # 512 chunks
NCORES = 8
NDT = D // 128   # 4 partition tiles of x / out
SB = 512         # Y-matmul sub-block (one PSUM bank of fp32)
CSB = SB // CS   # 32 chunk columns per sub-block
TBLOCKS = [(0, 512), (512, 1536), (2048, 2048), (4096, 2048),
           (6144, 1536), (7680, 512)]
TBMAX = 2048

_compiled_nc = None


def build_nc():
    import concourse.tile as tile
    from concourse import bacc, mybir

    f32 = mybir.dt.float32
    f16 = mybir.dt.float16
    AF = mybir.ActivationFunctionType
    ALU = mybir.AluOpType
    AX = mybir.AxisListType

    # Bacc (not plain Bass): its finalize() runs the TRN2 sync-wait
    # legalization (move_matmul_waits_to_ldweights / event-semaphore
    # splitting) that walrus codegen requires.
    nc = bacc.Bacc("TRN2", target_bir_lowering=False)
    x_d = nc.declare_dram_parameter("x", [D, T], f16, isOutput=False)
    w1t_d = nc.declare_dram_parameter("w1t", [D, DH], f16, isOutput=False)
    b1_d = nc.declare_dram_parameter("b1", [DH], f32, isOutput=False)
    w2t_d = nc.declare_dram_parameter("w2t", [DH, D], f16, isOutput=False)
    b2_d = nc.declare_dram_parameter("b2", [D], f32, isOutput=False)
    scale_d = nc.declare_dram_parameter("scale", [DH, TC], f32, isOutput=False)
    out_d = nc.declare_dram_parameter("out", [D, T], f16, isOutput=True)

    with tile.TileContext(nc) as tc:
        with (
            tc.tile_pool(name="xres", bufs=1) as xres,
            tc.tile_pool(name="small", bufs=1) as small,
            tc.tile_pool(name="ups", bufs=2) as ups,
            tc.tile_pool(name="psum_y", bufs=1, space="PSUM") as psum_y,
            tc.tile_pool(name="psum_g", bufs=4, space="PSUM") as psum_g,
        ):
            # x resident in SBUF: [128, 4, 8192] fp16 = 8 MB
            xt = xres.tile([128, NDT, T], f16, tag="x", name="x")
            w1s = small.tile([128, NDT, DH], f16, tag="w1")
            w2s = small.tile([DH, D], f16, tag="w2")
            b1s = small.tile([DH, 1], f32, tag="b1")
            b2s = small.tile([128, NDT], f32, tag="b2")
            scl = small.tile([DH, TC], f32, tag="scl")
            q = small.tile([DH, TC], f32, tag="q")      # per-chunk w1@x sums
            qs = small.tile([DH, TC], f32, tag="qs")    # causal prefix
            h32 = small.tile([DH, TC], f32, tag="h32")
            h16 = small.tile([DH, TC], f16, tag="h16")
            yp = [
                psum_y.tile([DH, SB], f32, tag=f"y{sb}", name=f"y{sb}")
                for sb in range(TBMAX // SB)
            ]

            # Dummy 1-element sigmoid: forces the walrus-inserted
            # ACT_TABLE_LOAD for the sigmoid set to run during the startup
            # DMA window instead of stalling ACT before the first real
            # sigmoid mid-stream.
            dummy = small.tile([1, 2], f32, tag="dummy")
            nc.gpsimd.memset(dummy[:], 0.0)
            nc.scalar.activation(dummy[:, 1:2], dummy[:, 0:1], AF.Sigmoid)

            # Loads alternate between the two HWDGE rings (ACT + SP) so the
            # per-transfer completion gaps of one ring hide under the other.
            # Tiny weight loads go first (everything downstream needs them);
            # stores are issued later on the SP ring (plus the ACT ring for
            # the tail, when it has gone idle).
            def load_block(eng, t0, TB):
                eng.dma_start(
                    xt[:, :, t0:t0 + TB],
                    x_d[:, t0:t0 + TB].rearrange("(k p) t -> p k t", p=128),
                )

            nc.scalar.dma_start(
                w1s[:], w1t_d[:].rearrange("(d p) h -> p d h", d=NDT)
            )
            nc.scalar.dma_start(b1s[:], b1_d[:].unsqueeze(1))
            nc.scalar.dma_start(scl[:], scale_d[:])
            nc.sync.dma_start(w2s[:], w2t_d[:])
            nc.sync.dma_start(
                b2s[:], b2_d[:].rearrange("(d p) -> p d", d=NDT)
            )
            for bi, (t0, TB) in enumerate(TBLOCKS):
                load_block(nc.scalar if bi % 2 == 0 else nc.sync, t0, TB)

            # Causal pipeline: gate for chunk c needs only x[:, :16(c+1)].
            # Block k's gate multiplies + stores are emitted AFTER block
            # k+1's reduce/scan stage so the in-order DVE queue never stalls
            # waiting for the ACT sigmoid-upsample (software pipelining).
            deferred = None

            def emit_mults(items, tail=False):
                for di, t0_, TB_, u_ in items:
                    xv = xt[:, di, t0_:t0_ + TB_]
                    nc.vector.tensor_tensor(xv, xv, u_[:, :TB_], op=ALU.mult)
                    if tail:
                        # tail: per-d-tile stores, alternating rings (the
                        # ACT ring is idle by now), right behind each
                        # multiply so the drain pipelines
                        deng = nc.sync if di < 2 else nc.scalar
                        deng.dma_start(
                            out_d[di * 128:(di + 1) * 128, t0_:t0_ + TB_], xv
                        )
                if not tail:
                    di, t0_, TB_, _ = items[0]
                    for half in range(2):
                        nc.sync.dma_start(
                            out_d[half * 256:(half + 1) * 256,
                                  t0_:t0_ + TB_].rearrange(
                                      "(k p) t -> p k t", p=128),
                            xt[:, 2 * half:2 * half + 2, t0_:t0_ + TB_],
                        )

            for tb, (t0, TB) in enumerate(TBLOCKS):
                CB = TB // CS
                c0 = t0 // CS
                nsb = TB // SB
                # Y = w1 @ x for this block: ki-outer so each stationary
                # w1-slice is reused across the sub-blocks while PSUM
                # accumulation groups interleave across banks.
                for ki in range(NDT):
                    for sb in range(nsb):
                        ts = t0 + sb * SB
                        nc.tensor.matmul(
                            yp[sb][:],
                            w1s[:, ki, :],
                            xt[:, ki, ts:ts + SB],
                            start=(ki == 0),
                            stop=(ki == NDT - 1),
                        )
                # chunk sums of Y: windowed reduce straight off PSUM
                for sb in range(nsb):
                    cc = c0 + sb * CSB
                    nc.vector.reduce_sum(
                        q[:, cc:cc + CSB],
                        yp[sb][:].rearrange("p (c j) -> p c j", j=CS),
                        axis=AX.X,
                    )
                # running causal prefix over this block (carry = last col)
                nc.vector.tensor_tensor_scan(
                    qs[:, c0:c0 + CB],
                    q[:, c0:c0 + CB],
                    q[:, c0:c0 + CB],
                    0.0 if tb == 0 else qs[:, c0 - 1:c0],
                    op0=ALU.add,
                    op1=ALU.bypass,
                )
                # h = relu(prefix * 1/(16(c+1)) + b1), relu on DVE so ACT
                # only ever needs the sigmoid table set
                nc.vector.tensor_mul(
                    h32[:, c0:c0 + CB], qs[:, c0:c0 + CB], scl[:, c0:c0 + CB]
                )
                nc.vector.tensor_scalar(
                    h16[:, c0:c0 + CB], h32[:, c0:c0 + CB],
                    b1s[:, :1], 0.0, op0=ALU.add, op1=ALU.max,
                )
                last = tb == len(TBLOCKS) - 1
                if last and deferred is not None:
                    # flush the previous block's multiplies first so the
                    # tail drains in order
                    emit_mults(deferred)
                    deferred = None
                cur = []
                for di in range(NDT):
                    gp = psum_g.tile([128, TBMAX // CS], f32, tag="g",
                                     name="gp")
                    nc.tensor.matmul(
                        gp[:, :CB],
                        w2s[:, di * 128:(di + 1) * 128],
                        h16[:, c0:c0 + CB],
                        start=True,
                        stop=True,
                    )
                    # fused sigmoid + 16x upsample: broadcast-read the
                    # PSUM column per chunk, write the dense fp16 gate
                    u = ups.tile(
                        [128, TBMAX], f16, tag=f"u{di}", name=f"u{di}"
                    )
                    nc.scalar.activation(
                        u[:, :TB].rearrange("p (c j) -> p c j", j=CS),
                        gp[:, :CB].unsqueeze(2).broadcast_to([128, CB, CS]),
                        AF.Sigmoid,
                        bias=b2s[:, di:di + 1],
                    )
                    if last:
                        # tail block: multiply right behind each sigmoid so
                        # the drain pipelines at d-tile granularity
                        emit_mults([(di, t0, TB, u)], tail=True)
                    else:
                        cur.append((di, t0, TB, u))
                if deferred is not None:
                    emit_mults(deferred)
                deferred = cur if not last else None
    # run_bass_via_pjrt serializes nc.m as-is; Bacc defers register
    # allocation and TRN2 sync-wait legalization to finalize(), so it must
    # run here or walrus rejects the BIR.
    nc.finalize()
    return nc


def _host_inputs(x, w1, b1, w2, b2, chunksize):
    x = np.asarray(x)
    w1 = np.asarray(w1, dtype=np.float32)
    b1 = np.ascontiguousarray(np.asarray(b1, dtype=np.float32))
    w2 = np.asarray(w2, dtype=np.float32)
    b2 = np.ascontiguousarray(np.asarray(b2, dtype=np.float32))
    cs = int(chunksize)
    assert cs == CS and x.shape == (B, D, T), (cs, x.shape)
    x16 = np.ascontiguousarray(x.astype(np.float16))
    w1t = np.ascontiguousarray(w1.T.astype(np.float16))      # [D, DH]
    w2t = np.ascontiguousarray(w2.T.astype(np.float16))      # [DH, D]
    scale = np.broadcast_to(
        1.0 / (CS * np.arange(1, TC + 1, dtype=np.float32)), (DH, TC)
    )
    scale = np.ascontiguousarray(scale)
    shared = dict(w1t=w1t, b1=b1, w2t=w2t, b2=b2, scale=scale)
    return x16, shared


def kernel(x, w1, b1, w2, b2, chunksize):
    global _compiled_nc
    from concourse.bass_utils import run_bass_kernel_spmd

    x16, shared = _host_inputs(x, w1, b1, w2, b2, chunksize)
    if _compiled_nc is None:
        _compiled_nc = build_nc()
    in_maps = [
        {"x": np.ascontiguousarray(x16[i]), **shared} for i in range(NCORES)
    ]
    res = run_bass_kernel_spmd(_compiled_nc, in_maps, list(range(NCORES)))
    out = np.stack(
        [res.results[i]["out"] for i in range(NCORES)], axis=0
    ).astype(np.float32)
    return out
